# revision 24
# baseline (speedup 1.0000x reference)
"""Bahdanau decoder kernel for Trainium2 (8 NeuronCores, SPMD).

Strategy:
  - batch-parallel: core c owns batch rows {2c, 2c+1} for the sequential
    recurrence (phases A+B); vocab projection is V-sharded (phase C) with an
    AllGather of the maxout output and an AllReduce of the softmax partial
    sums.
  - recurrence is done in "transposed" layout (features on partitions,
    batch on free dim) so gates/updates run at full DVE width and the next
    step's matmul operands need no transposes.
  - G = [Wq | W_hh^T] lives in SBUF as bf16 128x128 stationary tiles
    (fast-weight-load); h is fed as bf16 hi+lo pair so h stays fp32-exact.
    P (= enc @ W_c^T), we, tanh-energies stay fp32 (bf16 there flips argmax).
"""

import os
import numpy as np
from contextlib import ExitStack

import concourse.bass as bass
import concourse.mybir as mybir
import concourse.tile as tile
from concourse import bacc
from concourse.bass_utils import run_bass_kernel_spmd
from concourse.alu_op_type import AluOpType

F32 = mybir.dt.float32
BF16 = mybir.dt.bfloat16

B, T, S, H, E, V = 16, 128, 128, 1024, 512, 32000
NCORES = 8
BL = B // NCORES          # batch rows per core = 2
VSL = V // NCORES         # vocab slice per core = 4000
H3 = 3 * H
SLOT = 66                 # per-step loopbuf cols: 48 giY | 16 query | 2 E
QOFF = 48                 # query offset inside slot
EOFF = 64                 # energy offset inside slot
NSTEPS = int(os.environ.get("BAHDANAU_NSTEPS", str(T)))

Tanh = mybir.ActivationFunctionType.Tanh
Sigm = mybir.ActivationFunctionType.Sigmoid
Expf = mybir.ActivationFunctionType.Exp
Lnf = mybir.ActivationFunctionType.Ln


def build_nc(nsteps: int) -> bass.Bass:
    nc = bacc.Bacc()

    # ---- I/O ----
    G_in = nc.dram_tensor("G_in", [8, 128, 4096], BF16, kind="ExternalInput")
    Wi_in = nc.dram_tensor("Wi_in", [8, 128, H], F32, kind="ExternalInput")
    ehT_in = nc.dram_tensor("ehT_in", [8, 128, BL], F32, kind="ExternalInput")
    biT_in = nc.dram_tensor("biT_in", [128, 16], F32, kind="ExternalInput")
    Wk_in = nc.dram_tensor("Wk_in", [16, 128, H], F32, kind="ExternalInput")
    bkT_in = nc.dram_tensor("bkT_in", [128, 8], F32, kind="ExternalInput")
    WcT_in = nc.dram_tensor("WcT_in", [16, 128, H3], F32, kind="ExternalInput")
    WyT_in = nc.dram_tensor("WyT_in", [4, 128, H3], BF16, kind="ExternalInput")
    ysTb_in = nc.dram_tensor("ysTb_in", [BL, 4, 128, 128], BF16, kind="ExternalInput")
    ysTf_in = nc.dram_tensor("ysTf_in", [BL, 4, 128, 128], F32, kind="ExternalInput")
    bihT_in = nc.dram_tensor("bihT_in", [128, 24], F32, kind="ExternalInput")
    encT_in = nc.dram_tensor("encT_in", [BL, 16, 128, 128], F32, kind="ExternalInput")
    enc_in = nc.dram_tensor("enc_in", [BL, 128, 2 * H], F32, kind="ExternalInput")
    weT_in = nc.dram_tensor("weT_in", [128, 8], F32, kind="ExternalInput")
    gb_in = nc.dram_tensor("gb_in", [128, 64], F32, kind="ExternalInput")
    Wm_in = nc.dram_tensor("Wm_in", [28, 128, 2 * H], F32, kind="ExternalInput")
    bm_in = nc.dram_tensor("bm_in", [128, 2 * H], F32, kind="ExternalInput")
    Wp_in = nc.dram_tensor("Wp_in", [8, 128, VSL], F32, kind="ExternalInput")
    bp_in = nc.dram_tensor("bp_in", [128, VSL], F32, kind="ExternalInput")

    out_logits = nc.dram_tensor("out_logits", [B, 128, VSL], F32, kind="ExternalOutput")
    out_dect = nc.dram_tensor("out_dect", [128, nsteps * 16], F32, kind="ExternalOutput")
    out_dbg = nc.dram_tensor("out_dbg", [128, (nsteps + 1) * SLOT], F32, kind="ExternalOutput")

    # collective buffers (internal DRAM)
    preT_local = nc.dram_tensor("preT_local", [BL, 8, 128, 128], F32)
    preT_all = nc.dram_tensor("preT_all", [B, 8, 128, 128], F32, addr_space="Shared")
    esum_local = nc.dram_tensor("esum_local", [128, 16], F32)
    esum_all = nc.dram_tensor("esum_all", [128, 16], F32, addr_space="Shared")

    rg = [list(range(NCORES))]

    with TileCtx(nc) as tc, ExitStack() as ctx:
        persist = ctx.enter_context(tc.tile_pool(name="persist", bufs=1))

        # ------- persistent SBUF tensors -------
        loopbuf = persist.tile([128, (nsteps + 1) * SLOT], F32)
        dve_scr = persist.tile([128, 1], F32)

        def pe_touch(ap):
            # absorb a DMA-queue wait on the PE with a bare 1-col ldweights
            a = ap[:, 0:1]
            if a.dtype == F32:
                a = a.bitcast(BF16)
            nc.tensor.ldweights(a)

        def dve_touch(ap):
            nc.vector.tensor_copy(out=dve_scr, in_=ap[:, 0:1])

        abctx = ExitStack()
        pab = abctx.enter_context(tc.tile_pool(name="ab_scope", bufs=1))
        G_sb = pab.tile([128, 8 * 4096], BF16)              # 64KB/p
        hT = pab.tile([128, 32], BF16)                      # (kt8, b2, hi/lo)
        weT = pab.tile([128, 8], F32)
        gbias = pab.tile([128, 64], F32)
        bihT = pab.tile([128, 24], F32)

        nc.sync.dma_start(
            out=G_sb.rearrange("p (kt j) -> p kt j", kt=8),
            in_=G_in[:].rearrange("kt k j -> k kt j"))
        pe_touch(G_sb)
        nc.sync.dma_start(out=weT, in_=weT_in[:])
        pe_touch(weT)
        nc.sync.dma_start(out=gbias, in_=gb_in[:])
        dve_touch(gbias)
        nc.sync.dma_start(out=bihT, in_=bihT_in[:])
        dve_touch(bihT)

        # helper views of loopbuf
        def q_slot(t):      # query slot t (h_{t-1}) [128,16] fp32
            return loopbuf[:, t * SLOT + QOFF: t * SLOT + QOFF + 16]

        with ExitStack() as actx:
            pa = actx.enter_context(tc.tile_pool(name="phaseA", bufs=2))
            pa1 = actx.enter_context(tc.tile_pool(name="phaseA1", bufs=1))
            pap = actx.enter_context(tc.tile_pool(name="phaseA_ps", bufs=2, space="PSUM"))

            P_sb = pab.tile([128, BL * H3], F32)            # 24KB/p
            pkT = pab.tile([128, BL * 8 * 128], F32)        # 8KB/p

            encT_sb = pa1.tile([128, BL * 16 * 128], F32)   # 16KB/p
            nc.sync.dma_start(
                out=encT_sb.rearrange("p (b dt s) -> p b dt s", b=BL, dt=16),
                in_=encT_in[:].rearrange("b dt d s -> d b dt s"))
            pe_touch(encT_sb)
            ysTb_sb = pa1.tile([128, BL * 4 * 128], BF16)
            nc.sync.dma_start(
                out=ysTb_sb.rearrange("p (b et t) -> p b et t", b=BL, et=4),
                in_=ysTb_in[:].rearrange("b et e t -> e b et t"))
            pe_touch(ysTb_sb)
            ehT_sb = pa1.tile([128, 8 * BL], F32)
            nc.sync.dma_start(out=ehT_sb.rearrange("p (kt b) -> p kt b", kt=8),
                              in_=ehT_in[:].rearrange("kt k b -> k kt b"))
            pe_touch(ehT_sb)
            biT_sb = pa1.tile([128, 16], F32)
            nc.sync.dma_start(out=biT_sb, in_=biT_in[:])
            dve_touch(biT_sb)

            def encT(b, dt):
                return encT_sb[:, b * 2048 + dt * 128: b * 2048 + (dt + 1) * 128]

            # ---- h0 = tanh(enc_hidden @ Wi + bi), transposed layout ----
            # NB: accumulation groups must be contiguous per PSUM bank
            # (start=True invalidates the whole bank's has_written flags),
            # so loop hc-outer / kt-inner with streamed Wi slices.
            h0_ps = pap.tile([128, 16], F32, bufs=1)
            for hc in range(8):
                for kt in range(8):
                    wi_buf = pa.tile([128, 128], F32, tag="wi")
                    nc.sync.dma_start(out=wi_buf,
                                      in_=Wi_in[kt, :, hc * 128:(hc + 1) * 128])
                    pe_touch(wi_buf)
                    nc.tensor.matmul(
                        h0_ps[:, hc * 2: hc * 2 + 2],
                        wi_buf,
                        ehT_sb[:, kt * BL:(kt + 1) * BL],
                        start=(kt == 0), stop=(kt == 7))
            h0p = pa1.tile([128, 16], F32)
            nc.vector.tensor_tensor(h0p, h0_ps, biT_sb, AluOpType.add)
            nc.scalar.activation(q_slot(0), h0p, Tanh)
            split_h(nc, q_slot(0), hT, pa1)

            # ---- pkT[b] = (enc[b] @ Wk + bk)^T   [h-part, (hc, s)] ----
            for hc in range(8):
                pk_ps = [pap.tile([128, 128], F32, tag="pkps", name=f"pkps{_b}") for _b in range(BL)]
                for dt in range(16):
                    wk_t = pa.tile([128, 128], F32, tag="wk")
                    nc.sync.dma_start(out=wk_t, in_=Wk_in[dt, :, hc * 128:(hc + 1) * 128])
                    pe_touch(wk_t)
                    for b in range(BL):
                        nc.tensor.matmul(pk_ps[b], wk_t, encT(b, dt),
                                         start=(dt == 0), stop=(dt == 15))
                bk_col = pa1.tile([128, 1], F32, tag=f"bk{hc}")
                nc.sync.dma_start(out=bk_col, in_=bkT_in[:, hc:hc + 1])
                dve_touch(bk_col)
                for b in range(BL):
                    nc.vector.tensor_scalar(
                        pkT[:, b * 1024 + hc * 128: b * 1024 + (hc + 1) * 128],
                        pk_ps[b], bk_col, None, AluOpType.add)

            # ---- P[b] = enc[b] @ W_c^T   [s-part, j-free] fp32 ----
            for jc in range(6):
                P_ps = [pap.tile([128, 512], F32, tag="pps", name=f"pps{_b}") for _b in range(BL)]
                for dt in range(16):
                    wc_t = pa.tile([128, 512], F32, tag="wc")
                    nc.sync.dma_start(out=wc_t, in_=WcT_in[dt, :, jc * 512:(jc + 1) * 512])
                    pe_touch(wc_t)
                    for b in range(BL):
                        nc.tensor.matmul(P_ps[b], encT(b, dt), wc_t,
                                         start=(dt == 0), stop=(dt == 15))
                for b in range(BL):
                    nc.vector.tensor_copy(
                        out=P_sb[:, b * H3 + jc * 512: b * H3 + (jc + 1) * 512],
                        in_=P_ps[b])

            # ---- giY[t] = (ys[t] @ W_y^T + b_ih)^T  -> loopbuf slots ----
            for jc in range(24):
                gy_ps = [pap.tile([128, 128], F32, tag="gyps", name=f"gyps{_b}") for _b in range(BL)]
                for et in range(4):
                    wy_t = pa.tile([128, 128], BF16, tag="wy")
                    nc.sync.dma_start(out=wy_t, in_=WyT_in[et, :, jc * 128:(jc + 1) * 128])
                    pe_touch(wy_t)
                    for b in range(BL):
                        nc.tensor.matmul(
                            gy_ps[b], wy_t,
                            ysTb_sb[:, b * 512 + et * 128: b * 512 + (et + 1) * 128],
                            start=(et == 0), stop=(et == 3))
                lb3 = loopbuf.rearrange("p (t s) -> p t s", s=SLOT)
                for b in range(BL):
                    nc.vector.tensor_scalar(lb3[:, 0:nsteps, jc * 2 + b],
                                            gy_ps[b][:, :nsteps],
                                            bihT[:, jc:jc + 1], None, AluOpType.add)

        # =================== phase B: recurrence ===================
        with ExitStack() as bctx:
            pb = bctx.enter_context(tc.tile_pool(name="phaseB", bufs=2))
            pbp = bctx.enter_context(tc.tile_pool(name="phaseB_ps", bufs=1, space="PSUM"))

            giY_v = loopbuf[:, : nsteps * SLOT]

            def body(i):
                # i is pre-scaled by SLOT (loop step = SLOT)
                gT_pq = pbp.tile([128, 32], F32, tag="gtpq")      # (mc0..7, b2, hi/lo)
                gT_gh = pbp.tile([128, 96], F32, tag="gtgh")      # (mc8..31, b2, hi/lo)
                for mc in range(8):
                    for kt in range(8):
                        nc.tensor.matmul(
                            gT_pq[:, mc * 4:(mc + 1) * 4],
                            G_sb[:, (kt * 32 + mc) * 128:(kt * 32 + mc + 1) * 128],
                            hT[:, kt * 4:(kt + 1) * 4],
                            start=(kt == 0), stop=(kt == 7))
                gsum = pb.tile([128, 64], F32, tag="gsum")
                gsA = pb.tile([128, 64], F32, tag="gsA")
                gsB = pb.tile([128, 64], F32, tag="gsB")
                pq3 = gT_pq.rearrange("p (m two) -> p m two", two=2)
                nc.vector.tensor_copy(out=gsA[:, 0:16], in_=pq3[:, :, 0])
                nc.vector.tensor_tensor(gsB[:, 0:16], gsA[:, 0:16],
                                        pq3[:, :, 1], AluOpType.add)
                nc.vector.tensor_tensor(gsum[:, 0:16], gsB[:, 0:16],
                                        gbias[:, 0:16], AluOpType.add)
                for mc in range(8, 32):
                    for kt in range(8):
                        nc.tensor.matmul(
                            gT_gh[:, (mc - 8) * 4:(mc - 7) * 4],
                            G_sb[:, (kt * 32 + mc) * 128:(kt * 32 + mc + 1) * 128],
                            hT[:, kt * 4:(kt + 1) * 4],
                            start=(kt == 0), stop=(kt == 7))
                gh3 = gT_gh.rearrange("p (m two) -> p m two", two=2)
                nc.vector.tensor_copy(out=gsA[:, 16:64], in_=gh3[:, :, 0])
                nc.vector.tensor_tensor(gsB[:, 16:64], gsA[:, 16:64],
                                        gh3[:, :, 1], AluOpType.add)
                nc.vector.tensor_tensor(gsum[:, 16:64], gsB[:, 16:64],
                                        gbias[:, 16:64], AluOpType.add)

                # energy: v = tanh(pkT + pq), e = v^T @ we
                v_sb = pb.tile([128, BL * 1024], F32, tag="vsb")
                for b in range(BL):
                    for hc in range(8):
                        nc.scalar.activation(
                            v_sb[:, b * 1024 + hc * 128: b * 1024 + (hc + 1) * 128],
                            pkT[:, b * 1024 + hc * 128: b * 1024 + (hc + 1) * 128],
                            Tanh, bias=gsum[:, hc * 2 + b: hc * 2 + b + 1])
                e_ps = pbp.tile([128, BL], F32, tag="eps")
                for b in range(BL):
                    for hc in range(8):
                        nc.tensor.matmul(
                            e_ps[:, b:b + 1],
                            v_sb[:, b * 1024 + hc * 128: b * 1024 + (hc + 1) * 128],
                            weT[:, hc:hc + 1],
                            start=(hc == 0), stop=(hc == 7))
                e_st = pb.tile([128, BL], F32, tag="est")
                nc.vector.tensor_copy(out=e_st, in_=e_ps)
                ecols = loopbuf[:, EOFF:]
                nc.vector.tensor_copy(out=ecols[:, bass.ds(i, BL)], in_=e_st)

                # gi_c^T = P^T-chunks applied to e
                gic_ps = pbp.tile([128, 48], F32, tag="gic")
                for b in range(BL):
                    erhs = e_st[:, b:b + 1]
                    for jc in range(24):
                        nc.tensor.matmul(
                            gic_ps[:, jc * 2 + b: jc * 2 + b + 1],
                            P_sb[:, b * H3 + jc * 128: b * H3 + (jc + 1) * 128],
                            erhs, start=True, stop=True)

                # gates
                gi = pb.tile([128, 48], F32, tag="gi")
                nc.vector.tensor_tensor(gi, gic_ps, giY_v[:, bass.ds(i, 48)],
                                        AluOpType.add)
                rzp = pb.tile([128, 32], F32, tag="rzp")
                nc.vector.tensor_tensor(rzp, gi[:, 0:32], gsum[:, 16:48], AluOpType.add)
                rz = pb.tile([128, 32], F32, tag="rz")
                nc.scalar.activation(rz, rzp, Sigm)
                tmp16 = pb.tile([128, 16], F32, tag="t16")
                nc.vector.tensor_tensor(tmp16, rz[:, 0:16], gsum[:, 48:64],
                                        AluOpType.mult)
                tmp16b = pb.tile([128, 16], F32, tag="t16b")
                nc.vector.tensor_tensor(tmp16b, gi[:, 32:48], tmp16, AluOpType.add)
                n_sb = pb.tile([128, 16], F32, tag="nsb")
                nc.scalar.activation(n_sb, tmp16b, Tanh)
                hmn = pb.tile([128, 16], F32, tag="hmn")
                qv = loopbuf[:, QOFF:]
                nc.vector.tensor_tensor(hmn, qv[:, bass.ds(i, 16)], n_sb,
                                        AluOpType.subtract)
                hmn2 = pb.tile([128, 16], F32, tag="hmn2")
                nc.vector.tensor_tensor(hmn2, rz[:, 16:32], hmn, AluOpType.mult)
                qnext = loopbuf[:, QOFF + SLOT:]
                hnew = qnext[:, bass.ds(i, 16)]
                nc.vector.tensor_tensor(hnew, n_sb, hmn2, AluOpType.add)
                # split h into bf16 hi/lo for next step
                hi_v = hT.rearrange("p (m two) -> p m two", two=2)[:, :, 0]
                lo_v = hT.rearrange("p (m two) -> p m two", two=2)[:, :, 1]
                nc.vector.tensor_copy(out=hi_v, in_=hnew)
                hi_f = pb.tile([128, 16], F32, tag="hif")
                nc.vector.tensor_copy(out=hi_f, in_=hi_v)
                res = pb.tile([128, 16], F32, tag="res")
                nc.vector.tensor_tensor(res, hnew, hi_f, AluOpType.subtract)
                nc.vector.tensor_copy(out=lo_v, in_=res)

            if nsteps > 4:
                with tc.For_i(0, nsteps * SLOT, SLOT,
                              hint_engines=(mybir.EngineType.PE,)) as i:
                    body(i)
            else:
                for t in range(nsteps):
                    body(t * SLOT)

        abctx.close()

        # =================== phase C ===================
        with ExitStack() as cctx:

            lb3 = loopbuf.rearrange("p (t s) -> p t s", s=SLOT)
            # dec output: query slots 1..nsteps
            nc.sync.dma_start(out=out_dect[:],
                              in_=lb3[:, 1:nsteps + 1, QOFF:QOFF + 16])
            if int(os.environ.get("BAHDANAU_DEBUG", "0")):
                nc.sync.dma_start(out=out_dbg[:], in_=loopbuf)

            ev = [lb3[:, 0:nsteps, EOFF + b] for b in range(BL)]
            qv = [lb3[:, 0:nsteps, QOFF + kb] for kb in range(16)]

            # ctxT[b, dc] = enc[b][:, dc]^T @ E[b]
            pcB = ExitStack()
            pB = pcB.enter_context(tc.tile_pool(name="phaseCB", bufs=1))
            pcA = ExitStack()
            pA = pcA.enter_context(tc.tile_pool(name="phaseCA", bufs=1))
            enc_sb = pA.tile([128, BL * 2 * H], F32)
            nc.sync.dma_start(out=enc_sb.rearrange("p (b d) -> p b d", b=BL),
                              in_=enc_in[:].rearrange("b s d -> s b d"))
            pe_touch(enc_sb)
            ctxT = pB.tile([128, BL * 16 * 128], F32)
            with tc.tile_pool(name="ctx_ps", bufs=4, space="PSUM") as pcp_c:
                for b in range(BL):
                    for dc in range(16):
                        c_ps = pcp_c.tile([128, 128], F32, tag="cps")
                        nc.tensor.matmul(
                            c_ps[:, :nsteps],
                            enc_sb[:, b * 2048 + dc * 128: b * 2048 + (dc + 1) * 128],
                            ev[b], start=True, stop=True)
                        nc.vector.tensor_copy(
                            out=ctxT[:, b * 2048 + dc * 128:
                                     b * 2048 + dc * 128 + nsteps],
                            in_=c_ps[:, :nsteps])

            pcA.close()
            ysTf_sb = pB.tile([128, BL * 4 * 128], F32)
            nc.sync.dma_start(
                out=ysTf_sb.rearrange("p (b et t) -> p b et t", b=BL, et=4),
                in_=ysTf_in[:].rearrange("b et e t -> e b et t"))
            pe_touch(ysTf_sb)
            bm_sb = pB.tile([128, 2 * H], F32)
            nc.sync.dma_start(out=bm_sb, in_=bm_in[:])
            dve_touch(bm_sb)

            def cat_lhsT(b, kt):
                if kt < 8:
                    return qv[kt * 2 + b]
                if kt < 24:
                    dc = kt - 8
                    return ctxT[:, b * 2048 + dc * 128: b * 2048 + dc * 128 + nsteps]
                et = kt - 24
                return ysTf_sb[:, b * 512 + et * 128: b * 512 + et * 128 + nsteps]

            # maxout -> pre [t-part, 1024] per b
            pBs = pcB.enter_context(tc.tile_pool(name="phaseCBs", bufs=1))
            pre_sb = pB.tile([128, BL * H], F32)
            with tc.tile_pool(name="mx_ps", bufs=1, space="PSUM") as pcp_m:
                tt_ps = [[pcp_m.tile([128, 512], F32, tag=f"tt{b}_{n}", name=f"tt{b}_{n}")
                          for n in range(4)] for b in range(BL)]
                for kt in range(28):
                    wm_t = pBs.tile([128, 2 * H], F32, tag="wm", bufs=2)
                    nc.sync.dma_start(out=wm_t, in_=Wm_in[kt])
                    pe_touch(wm_t)
                    for b in range(BL):
                        lh = cat_lhsT(b, kt)
                        for n in range(4):
                            nc.tensor.matmul(tt_ps[b][n][:nsteps, :], lh,
                                             wm_t[:, n * 512:(n + 1) * 512],
                                             start=(kt == 0), stop=(kt == 27))
                for b in range(BL):
                    for n in range(4):
                        tb = pBs.tile([128, 512], F32, tag="ttsb", bufs=2)
                        nc.vector.tensor_tensor(tb[:nsteps, :],
                                                tt_ps[b][n][:nsteps, :],
                                                bm_sb[:nsteps, n * 512:(n + 1) * 512],
                                                AluOpType.add)
                        po = pre_sb[:, b * H + n * 256: b * H + (n + 1) * 256]
                        tb2 = tb.rearrange("p (j two) -> p j two", two=2)
                        nc.vector.tensor_tensor(po[:nsteps, :],
                                                tb2[:nsteps, :, 0],
                                                tb2[:nsteps, :, 1], AluOpType.max)

            # transpose pre -> preT, DMA to DRAM, AllGather
            from concourse.masks import make_identity
            ident = pB.tile([128, 128], F32)
            make_identity(nc, ident)
            pe_touch(ident)
            pe_touch(ctxT)
            preT_stage = pB.tile([128, BL * 8 * 128], F32)
            with tc.tile_pool(name="tr_ps", bufs=2, space="PSUM") as pcp_t:
                for b in range(BL):
                    for kt in range(8):
                        tr_ps = pcp_t.tile([128, 128], F32, tag="trps")
                        nc.tensor.transpose(tr_ps,
                                            pre_sb[:, b * H + kt * 128:
                                                   b * H + (kt + 1) * 128], ident)
                        nc.vector.tensor_copy(
                            out=preT_stage[:, (b * 8 + kt) * 128:
                                           (b * 8 + kt + 1) * 128],
                            in_=tr_ps)
            nc.sync.dma_start(
                out=preT_local[:].rearrange("b kt j t -> j b kt t"),
                in_=preT_stage.rearrange("p (b kt t) -> p b kt t", b=BL, kt=8))

            pcB.close()
            nc.gpsimd.collective_compute(
                "AllGather", AluOpType.bypass, replica_groups=rg,
                ins=[preT_local[:]], outs=[preT_all[:]])

            # logits over V-slice for ALL b; stream Wp once
            pD = cctx.enter_context(tc.tile_pool(name="phaseCD", bufs=1))
            preT_sb = pD.tile([128, B * 8 * 128], F32)      # 64KB/p
            pa_ap = bass.AP(tensor=preT_all, offset=0,
                            ap=[[128, 128], [8 * 128 * 128, 16], [128 * 128, 8],
                                [1, 128]])
            nc.sync.dma_start(out=preT_sb, in_=pa_ap)
            pe_touch(preT_sb)
            bp_sb = pD.tile([128, VSL], F32)
            nc.sync.dma_start(out=bp_sb, in_=bp_in[:])
            dve_touch(bp_sb)
            exps = pD.tile([128, B * 8], F32)
            scratch = pD.tile([128, 512], F32)
            nc.vector.memset(exps, 0.0)

            pcp_l = cctx.enter_context(
                tc.tile_pool(name="lg_ps", bufs=4, space="PSUM"))
            NCW = [512] * 7 + [VSL - 7 * 512]
            for ncid in range(8):
                off = ncid * 512
                w = NCW[ncid]
                wp_big = pD.tile([128, 8 * 512], F32, tag="wpb", bufs=2)
                nc.sync.dma_start(
                    out=wp_big.rearrange("p (kt v) -> p kt v", kt=8)[:, :, :w],
                    in_=Wp_in[:, :, off:off + w].rearrange("kt k v -> k kt v"))
                pe_touch(wp_big)
                wp_t = [wp_big[:, kt * 512: kt * 512 + w] for kt in range(8)]
                for bg in range(B):
                    lg_ps = pcp_l.tile([128, 512], F32, tag="lgps")
                    for kt in range(8):
                        nc.tensor.matmul(
                            lg_ps[:nsteps, :w],
                            preT_sb[:, (bg * 8 + kt) * 128:(bg * 8 + kt + 1) * 128][:, :nsteps],
                            wp_t[kt],
                            start=(kt == 0), stop=(kt == 7))
                    lg_sb = pD.tile([128, 512], F32, tag="lgsb", bufs=2)
                    nc.vector.tensor_tensor(lg_sb[:nsteps, :w], lg_ps[:nsteps, :w],
                                            bp_sb[:nsteps, off:off + w], AluOpType.add)
                    nc.scalar.activation(scratch[:nsteps, :w], lg_sb[:nsteps, :w], Expf,
                                         accum_out=exps[:nsteps, bg * 8 + ncid:
                                                        bg * 8 + ncid + 1])
                    nc.sync.dma_start(out=out_logits[bg, :nsteps, off:off + w],
                                      in_=lg_sb[:nsteps, :w])

            esum_sb = pD.tile([128, 16], F32)
            for bg in range(B):
                nc.vector.reduce_sum(esum_sb[:nsteps, bg:bg + 1],
                                     exps[:nsteps, bg * 8:(bg + 1) * 8],
                                     mybir.AxisListType.X)
            nc.sync.dma_start(out=esum_local[:], in_=esum_sb)
            nc.gpsimd.collective_compute(
                "AllReduce", AluOpType.add, replica_groups=rg,
                ins=[esum_local[:]], outs=[esum_all[:]])

            lse_raw = pD.tile([128, 16], F32)
            nc.sync.dma_start(out=lse_raw, in_=esum_all[:])
            lse_sb = pD.tile([128, 16], F32)
            nc.scalar.activation(lse_sb, lse_raw, Lnf)

            # pass 2: logp = logits - lse
            for bg in range(B):
                for ncid in range(8):
                    off = ncid * 512
                    w = NCW[ncid]
                    lb = pD.tile([128, 512], F32, tag="lb2", bufs=2)
                    nc.sync.dma_start(out=lb[:nsteps, :w],
                                      in_=out_logits[bg, :nsteps, off:off + w])
                    dve_touch(lb)
                    lbo = pD.tile([128, 512], F32, tag="lbo", bufs=2)
                    nc.vector.tensor_scalar(lbo[:nsteps, :w], lb[:nsteps, :w],
                                            lse_sb[:nsteps, bg:bg + 1], None,
                                            AluOpType.subtract)
                    nc.sync.dma_start(out=out_logits[bg, :nsteps, off:off + w],
                                      in_=lbo[:nsteps, :w])
    return nc


def split_h(nc, h_f32, hT, pool):
    hi_v = hT.rearrange("p (m two) -> p m two", two=2)[:, :, 0]
    lo_v = hT.rearrange("p (m two) -> p m two", two=2)[:, :, 1]
    nc.vector.tensor_copy(out=hi_v, in_=h_f32)
    hi_f = pool.tile([128, 16], F32, tag="hif0")
    nc.vector.tensor_copy(out=hi_f, in_=hi_v)
    res = pool.tile([128, 16], F32, tag="res0")
    nc.vector.tensor_tensor(res, h_f32, hi_f, AluOpType.subtract)
    nc.vector.tensor_copy(out=lo_v, in_=res)


def TileCtx(nc):
    return tile.TileContext(nc)


# ---------------- host side ----------------

def _prep_core_inputs(inputs, c, nsteps):
    f32 = np.float32
    bf16 = np.dtype("bfloat16") if hasattr(np, "bfloat16") else None
    import ml_dtypes
    bf16 = ml_dtypes.bfloat16

    y_ids = np.asarray(inputs["y_ids"])
    enc = np.asarray(inputs["enc_output"], f32)
    ench = np.asarray(inputs["enc_hidden"], f32)
    Wq = np.asarray(inputs["Wq"], f32); bq = np.asarray(inputs["bq"], f32)
    Wk = np.asarray(inputs["Wk"], f32); bk = np.asarray(inputs["bk"], f32)
    we = np.asarray(inputs["we"], f32)
    Wi = np.asarray(inputs["Wi"], f32); bi = np.asarray(inputs["bi"], f32)
    emb = np.asarray(inputs["emb"], f32)
    W_ih = np.asarray(inputs["W_ih"], f32); b_ih = np.asarray(inputs["b_ih"], f32)
    b_hh = np.asarray(inputs["b_hh"], f32)
    W_hh = np.asarray(inputs["W_hh"], f32)
    Wm = np.asarray(inputs["Wm"], f32); bm = np.asarray(inputs["bm"], f32)
    Wp = np.asarray(inputs["Wp"], f32); bp = np.asarray(inputs["bp"], f32)

    bsl = slice(c * BL, (c + 1) * BL)
    vsl = slice(c * VSL, (c + 1) * VSL)

    G = np.concatenate([Wq, W_hh.T], axis=1)              # [1024, 4096]
    d = {}
    d["G_in"] = np.ascontiguousarray(G.reshape(8, 128, 4096)).astype(bf16)
    d["Wi_in"] = np.ascontiguousarray(Wi.reshape(8, 128, H))
    d["ehT_in"] = np.ascontiguousarray(
        ench[0][bsl].T.reshape(8, 128, BL))
    biT = np.stack([bi.reshape(8, 128).T] * BL, -1).reshape(128, 16)
    d["biT_in"] = np.ascontiguousarray(biT)
    d["Wk_in"] = np.ascontiguousarray(Wk.reshape(16, 128, H))
    d["bkT_in"] = np.ascontiguousarray(bk.reshape(8, 128).T)
    W_c = W_ih[:, E:]                                      # [3H, 2H]
    d["WcT_in"] = np.ascontiguousarray(W_c.T.reshape(16, 128, H3))
    W_y = W_ih[:, :E]
    d["WyT_in"] = np.ascontiguousarray(W_y.T.reshape(4, 128, H3)).astype(bf16)
    ys = emb[y_ids[bsl]]                                   # [BL, T, E]
    ysT = np.ascontiguousarray(np.swapaxes(ys, 1, 2)).reshape(BL, 4, 128, T)
    ysT = ysT[:, :, :, :128]
    d["ysTb_in"] = ysT.astype(bf16)
    d["ysTf_in"] = np.ascontiguousarray(ysT.astype(f32))
    bih = b_ih.reshape(24, 128).T
    d["bihT_in"] = np.ascontiguousarray(bih)
    encc = enc[bsl]
    d["encT_in"] = np.ascontiguousarray(
        np.swapaxes(encc, 1, 2).reshape(BL, 16, 128, 128))
    d["enc_in"] = np.ascontiguousarray(encc)
    d["weT_in"] = np.ascontiguousarray(we.reshape(8, 128).T)
    gbias_vec = np.concatenate([bq, b_hh])                 # [4096]
    gb = np.stack([gbias_vec.reshape(32, 128).T] * 2, -1).reshape(128, 64)
    d["gb_in"] = np.ascontiguousarray(gb)
    d["Wm_in"] = np.ascontiguousarray(Wm.reshape(28, 128, 2 * H))
    d["bm_in"] = np.ascontiguousarray(np.broadcast_to(bm, (128, 2 * H)))
    d["Wp_in"] = np.ascontiguousarray(Wp[:, vsl].reshape(8, 128, VSL))
    d["bp_in"] = np.ascontiguousarray(np.broadcast_to(bp[vsl], (128, VSL)))
    return d


def kernel(**inputs):
    nsteps = NSTEPS
    nc = build_nc(nsteps)
    nc.compile()
    in_maps = [_prep_core_inputs(inputs, c, nsteps) for c in range(NCORES)]
    res = run_bass_kernel_spmd(nc, in_maps, core_ids=list(range(NCORES)))
    results = res.results

    dec = np.zeros((B, nsteps, H), np.float32)
    logp = np.zeros((B, nsteps, V), np.float32)
    for c in range(NCORES):
        r = results[c]
        dect = np.asarray(r["out_dect"], np.float32).reshape(128, nsteps, 8, BL)
        for b in range(BL):
            dec[c * BL + b] = np.transpose(dect[:, :, :, b], (1, 2, 0)).reshape(nsteps, H)
        lg = np.asarray(r["out_logits"], np.float32)[:, :nsteps, :]
        logp[:, :, c * VSL:(c + 1) * VSL] = lg
    preds = logp.argmax(-1).astype(np.int32)
    return dec, logp, preds
